# Initial kernel scaffold
#
"""Multi-head attention Trainium2 kernel (8 NeuronCores, SPMD).

Problem: nn_MultiHeadAttention (B=2, S=2048, E=1024, H=16, hd=64), fp32 I/O,
reference masks j <= i with -1e9 (attends strictly to the future); row S-1
degenerates to uniform attention and is patched on host.

Sharding: core c handles batch b = c//4 and 4 heads [4*(c%4), 4*(c%4)+4).
Each core computes a partial output [S, E] (its heads' contribution to the
output projection); host sums the 4 partials per batch and adds the bias.

Design (~153 us vs 359 us fp32r baseline; evolved over five traced
iterations):
  - all matmuls fp16 (measured 216 ns issue-to-issue for N=512 chains = 1
    col/cycle warm, vs ~2.5 cycles/col for the fp32 HIGH-mode path, which
    also serializes its weight loads). End-to-end error ~6e-4 of scale.
  - zero bias matmuls: k-bias dropped (softmax is invariant to per-query
    constants), q-bias fused into the qT PSUM->SBUF eviction as a DVE
    broadcast add, v-bias folded into b_out on host (attn rows sum to 1).
  - scores per head-pair: two K=64 matmuls on array row-groups 0/64;
    attn@v col-packed: head X -> PSUM partitions 64X..64X+63, concurrent
    via col groups (col-tiled matmuls get extra XBUS streams; row-tiled
    ones share one stream and serialize - measured, not in the docs).
  - softmax denominators as M=64 ones-matmuls writing 64 identical rows
    per head into dn partitions 0-63/64-127 (two concurrent col-tiled
    matmuls): the partition broadcast IS the matmul, so normalize is just
    reciprocal_approx_fast + one multiply, both partition-aligned with va.
  - exp on the scalar engine reads score pairs packed [0 : w0+w1] in one
    PSUM tile (no junk columns); diagonal masking is one fused dual-window
    DVE add using a stride-0 repeat of a [128,128] mask constant.
  - schedule: i-chunks processed 3->0 with jb descending; projection and
    out-proj units drip into the attention stream as fillers so PE work
    covers the scalar engine's exp; PSUM = scores ring 3x[128,1024] +
    va + dn (8 banks exactly).
  - DMA: packed [128, 8*X] input layouts, issue split across the Sync and
    Scalar HW-DGE queues, xt loaded in descending column quarters (q3
    per-e so the first chains pace with arrival); fp16 [128,1024] output
    stores; flush evictions/stores split across engines and queues.
"""

from collections import deque

import numpy as np

import concourse.bacc as bacc
import concourse.bass as bass
import concourse.mybir as mybir
from concourse.tile import TileContext
from concourse.bass_utils import run_bass_kernel_spmd

S = 2048
E = 1024
HD = 64
NH = 16
B = 2
NCORES = 8
NEG = -1000000000.0

F32 = mybir.dt.float32
F16 = mybir.dt.float16

EXP = mybir.ActivationFunctionType.Exp


def build_bass():
    nc = bacc.Bacc()

    xt_d = nc.dram_tensor("xt", [128, 8 * S], F16, kind="ExternalInput")
    wq_d = nc.dram_tensor("wq", [128, 2048], F16, kind="ExternalInput")
    wk_d = nc.dram_tensor("wk", [128, 2048], F16, kind="ExternalInput")
    wv_d = nc.dram_tensor("wv", [128, 2048], F16, kind="ExternalInput")
    wout_d = nc.dram_tensor("wout", [128, 2048], F16, kind="ExternalInput")
    cf_d = nc.dram_tensor("cf", [128, 130], F32, kind="ExternalInput")
    onesp_d = nc.dram_tensor("onesp", [128, 64], F16, kind="ExternalInput")
    out_d = nc.dram_tensor("out", [S, E], F16, kind="ExternalOutput")

    with TileContext(nc) as tc:
        with (
            tc.tile_pool(name="big", bufs=1) as big,
            tc.tile_pool(name="small", bufs=1) as small,
            tc.tile_pool(name="work", bufs=1) as work,
            tc.tile_pool(name="psum", bufs=1, space="PSUM") as pp,
        ):
            # ---- static SBUF tensors ----
            xt = big.tile([128, 8 * S], F16, tag="xt", name="xt")
            wq = small.tile([128, 2048], F16, tag="wq", name="wq")
            wk = small.tile([128, 2048], F16, tag="wk", name="wk")
            wv = small.tile([128, 2048], F16, tag="wv", name="wv")
            wout = small.tile([128, 2048], F16, tag="wout", name="wout")
            cf = small.tile([128, 130], F32, tag="cf", name="cf")
            onesp = small.tile([128, 64], F16, tag="onesp", name="onesp")
            # qT/kT tiles [128 = 2 heads x 64 dims, S]; index = 2*pair + qk
            qkT = [big.tile([128, S], F16, tag="qkT", bufs=4, name=f"qkT{i}")
                   for i in range(4)]
            # v natural [128 seq, 256 = 4 heads x 64]
            v16 = [big.tile([128, 256], F16, tag="v16", bufs=16,
                            name=f"v16_{s}") for s in range(16)]
            # normalized valsT per pair [128 dims, S]
            valsT = [big.tile([128, S], F16, tag="valsT", bufs=2,
                              name=f"valsT{t}") for t in range(2)]

            mask = cf[:, 0:128]

            # ---- input DMAs: issue split across the two HW-DGE queues
            # (Sync + Scalar) so issue serialization doesn't gate the
            # first projection chain; xt quarters descending, halves of
            # each quarter on different queues
            def xt_quarter(quarter, elo, ehi, eng):
                src = bass.AP(tensor=xt_d, offset=S * elo + 512 * quarter,
                              ap=[[8 * S, 128], [S, ehi - elo], [1, 512]])
                dst = bass.AP(tensor=xt.tensor,
                              offset=xt.offset + S * elo + 512 * quarter,
                              ap=[[xt.ap[0][0], 128], [S, ehi - elo],
                                  [1, 512]])
                eng.dma_start(out=dst, in_=src)

            # wq/wk are packed p-major ([*, 1024p + 128e .. +128]) so small
            # slices unblock the first projection chain; xt quarter 3 per-e
            # so each accumulation step starts as its chunk lands
            nc.sync.dma_start(out=cf, in_=cf_d[:, :])
            nc.sync.dma_start(out=onesp, in_=onesp_d[:, :])
            nc.scalar.dma_start(out=wq[:, 0:512], in_=wq_d[:, 0:512])
            nc.sync.dma_start(out=wk[:, 0:512], in_=wk_d[:, 0:512])
            for e in range(0, 8, 2):
                xt_quarter(3, e, e + 1, nc.scalar)
                xt_quarter(3, e + 1, e + 2, nc.sync)
            # second wq/wk halves feed the p=1 filler chains ~20us later;
            # keeping them out of the quarter-3 stream avoids delaying the
            # eager chains' per-e steps
            nc.scalar.dma_start(out=wq[:, 512:1024], in_=wq_d[:, 512:1024])
            nc.sync.dma_start(out=wk[:, 512:1024], in_=wk_d[:, 512:1024])
            nc.scalar.dma_start(out=wv, in_=wv_d[:, :])
            nc.sync.dma_start(out=wq[:, 1024:2048], in_=wq_d[:, 1024:2048])
            nc.sync.dma_start(out=wk[:, 1024:2048], in_=wk_d[:, 1024:2048])
            xt_quarter(2, 0, 4, nc.sync)
            xt_quarter(2, 4, 8, nc.scalar)
            xt_quarter(1, 0, 8, nc.sync)
            nc.sync.dma_start(out=wout, in_=wout_d[:, :])
            xt_quarter(0, 0, 8, nc.sync)

            # ---- projection / out-proj unit emitters (fillers) ----
            def q_chain(p, sc):
                def emit():
                    psb = pp.tile([128, 1024], F32, tag="sc", bufs=3,
                                  name=f"psq{p}{sc}")
                    ps = psb[:, 0:512]
                    for e in range(8):
                        nc.tensor.matmul(
                            ps, wq[:, 1024 * p + 128 * e:1024 * p + 128 * e + 128],
                            xt[:, 2048 * e + 512 * sc:2048 * e + 512 * sc + 512],
                            start=(e == 0), stop=(e == 7))
                    bcol = cf[:, 128 + p:129 + p]
                    bb = bass.AP(tensor=bcol.tensor, offset=bcol.offset,
                                 ap=[bcol.ap[0], [0, 512]])
                    nc.vector.tensor_add(
                        out=qkT[2 * p][:, 512 * sc:512 * sc + 512],
                        in0=ps, in1=bb)
                return emit

            def k_chain(p, sc):
                def emit():
                    psb = pp.tile([128, 1024], F32, tag="sc", bufs=3,
                                  name=f"psk{p}{sc}")
                    ps = psb[:, 0:512]
                    for e in range(8):
                        nc.tensor.matmul(
                            ps, wk[:, 1024 * p + 128 * e:1024 * p + 128 * e + 128],
                            xt[:, 2048 * e + 512 * sc:2048 * e + 512 * sc + 512],
                            start=(e == 0), stop=(e == 7))
                    nc.vector.tensor_copy(
                        out=qkT[2 * p + 1][:, 512 * sc:512 * sc + 512],
                        in_=ps)
                return emit

            def v_chain(sb):
                def emit():
                    psb = pp.tile([128, 1024], F32, tag="sc", bufs=3,
                                  name=f"psv{sb}")
                    ps = psb[:, 0:256]
                    for e in range(8):
                        nc.tensor.matmul(
                            ps, xt[:, 2048 * e + 128 * sb:2048 * e + 128 * sb + 128],
                            wv[:, 256 * e:256 * e + 256],
                            start=(e == 0), stop=(e == 7))
                    nc.vector.tensor_copy(out=v16[sb], in_=ps)
                return emit

            def outproj_unit(sb, use_act=False, dma_eng=None):
                def emit():
                    psb = pp.tile([128, 1024], F32, tag="sc", bufs=3,
                                  name=f"pso{sb}")
                    for ec in range(2):
                        for t in range(2):
                            nc.tensor.matmul(
                                psb[:, 512 * ec:512 * ec + 512],
                                valsT[t][:, 128 * sb:128 * sb + 128],
                                wout[:, 1024 * t + 512 * ec:
                                     1024 * t + 512 * ec + 512],
                                start=(t == 0), stop=(t == 1))
                    ob = work.tile([128, 1024], F16, tag="ob", bufs=4,
                                   name=f"ob{sb}")
                    if use_act:
                        nc.scalar.copy(ob, psb)
                    else:
                        nc.vector.tensor_copy(out=ob, in_=psb)
                    (dma_eng or nc.sync).dma_start(
                        out=out_d[128 * sb:128 * sb + 128, :], in_=ob)
                return emit

            fillers = deque()

            def emit_fillers(n):
                for _ in range(n):
                    if fillers:
                        fillers.popleft()()

            # eager: just enough projection for attention (ic3, p0) pair 0;
            # the rest of ic3's inputs lead the ic3 filler queue
            q_chain(0, 3)()
            k_chain(0, 3)()
            v_chain(15)()
            v_chain(14)()

            # ---- attention, ic descending; fillers drip into the stream ----
            for ic in (3, 2, 1, 0):
                if ic == 3:
                    newf = [v_chain(13), v_chain(12),
                            q_chain(1, 3), k_chain(1, 3)]
                    newf += [q_chain(p, 2) for p in range(2)]
                    newf += [k_chain(p, 2) for p in range(2)]
                    newf += [v_chain(sb) for sb in range(8, 12)]
                elif ic == 2:
                    newf = [q_chain(p, 1) for p in range(2)]
                    newf += [k_chain(p, 1) for p in range(2)]
                    newf += [v_chain(sb) for sb in range(4, 8)]
                    newf += [outproj_unit(sb) for sb in range(12, 16)]
                elif ic == 1:
                    newf = [q_chain(p, 0) for p in range(2)]
                    newf += [outproj_unit(sb) for sb in range(8, 12)]
                else:
                    newf = [k_chain(p, 0) for p in range(2)]
                    newf += [v_chain(sb) for sb in range(0, 4)]
                    newf += [outproj_unit(sb) for sb in range(4, 8)]
                fillers.extend(newf)

                i0 = 512 * ic
                for p in range(2):
                    qTp = qkT[2 * p]
                    kTp = qkT[2 * p + 1]
                    jbs = list(range(15, 4 * ic - 1, -1))  # descending
                    pairs = [(jbs[2 * i], jbs[2 * i + 1])
                             for i in range(len(jbs) // 2)]
                    first_jb = jbs[0]
                    last_jb = jbs[-1]
                    va = pp.tile([128, 512], F32, tag="va", bufs=1,
                                 name=f"va{p}{ic}")
                    dn = pp.tile([128, 512], F32, tag="dn", bufs=1,
                                 name=f"dn{p}{ic}")

                    def emit_attnv(jbpair, pts, widths, offs):
                        for half, jb in enumerate(jbpair):
                            w = widths[half]
                            o = offs[half]
                            st = (jb == first_jb)
                            sp = (jb == last_jb)
                            for X in range(2):
                                nc.tensor.matmul(
                                    va[64 * X:64 * X + 64, 0:w],
                                    v16[jb][:, 64 * (2 * p + X):
                                            64 * (2 * p + X) + 64],
                                    pts[X][:, o:o + w],
                                    start=st, stop=sp)
                            # denominators: 64 identical rows per head so
                            # the partition broadcast IS the matmul
                            for X in range(2):
                                nc.tensor.matmul(
                                    dn[64 * X:64 * X + 64, 0:w],
                                    onesp[:, 0:64],
                                    pts[X][:, o:o + w],
                                    start=st, stop=sp)

                    prev = None
                    for jb0, jb1 in pairs:
                        sc_ps = [pp.tile([128, 1024], F32, tag="sc", bufs=3,
                                         name=f"sc{p}{ic}{jb0}{X}")
                                 for X in range(2)]
                        pts = [work.tile([128, 1024], F16, tag="pt", bufs=4,
                                         name=f"pt{p}{ic}{jb0}{X}")
                               for X in range(2)]
                        # half 1 packed right after half 0 (cols w0..w0+w1):
                        # the exp call covers exactly w0+w1 useful columns
                        w0 = min(128 * jb0 - i0 + 128, 512)
                        w1 = min(128 * jb1 - i0 + 128, 512)
                        ws = [w0, w1]
                        offs = [0, w0]
                        for half, jb in enumerate((jb0, jb1)):
                            j0 = 128 * jb
                            for X in range(2):
                                nc.tensor.matmul(
                                    sc_ps[X][:, offs[half]:offs[half] + ws[half]],
                                    kTp[64 * X:64 * X + 64, j0:j0 + 128],
                                    qTp[64 * X:64 * X + 64, i0:i0 + ws[half]],
                                    start=True, stop=True)
                        o0 = 128 * jb0 - i0
                        if o0 <= 384:
                            # both halves diagonal: one fused add over the
                            # two 128-wide windows (w0-128 apart); the mask
                            # operand repeats via a stride-0 free dim
                            for X in range(2):
                                reg = sc_ps[X][:, o0:o0 + 128]
                                dual = bass.AP(
                                    tensor=reg.tensor, offset=reg.offset,
                                    ap=[reg.ap[0], [w0 - 128, 2], [1, 128]])
                                m2 = bass.AP(
                                    tensor=mask.tensor, offset=mask.offset,
                                    ap=[mask.ap[0], [0, 2], [1, 128]])
                                nc.vector.tensor_add(
                                    out=dual, in0=dual, in1=m2)
                        for X in range(2):
                            nc.scalar.activation(
                                pts[X][:, 0:w0 + w1],
                                sc_ps[X][:, 0:w0 + w1], EXP)
                        if prev is not None:
                            emit_attnv(*prev)
                        emit_fillers(2 if ic >= 2 else 1)
                        prev = ((jb0, jb1), pts, ws, offs)
                    # one filler covers the last pair's exp latency (no
                    # further scores follow to keep the PE busy here)
                    emit_fillers(1)
                    if prev is not None:
                        emit_attnv(*prev)

                    # normalize: reciprocal of the (already broadcast)
                    # denominators, then one partition-aligned multiply
                    rcpt = work.tile([128, 512], F32, tag="rcpt", bufs=2,
                                     name=f"rcpt{p}{ic}")
                    nc.vector.reciprocal_approx_fast(out=rcpt, in_=dn)
                    nc.vector.tensor_mul(
                        out=valsT[p][:, i0:i0 + 512], in0=va, in1=rcpt)

            # flush remaining fillers + final out-proj chunk: evictions
            # split per-half across Scalar/Vector with dual-queue DMAs to
            # compress the tail
            emit_fillers(len(fillers))
            for sb in range(0, 4):
                psb = pp.tile([128, 1024], F32, tag="sc", bufs=3,
                              name=f"psf{sb}")
                for ec in range(2):
                    for t in range(2):
                        nc.tensor.matmul(
                            psb[:, 512 * ec:512 * ec + 512],
                            valsT[t][:, 128 * sb:128 * sb + 128],
                            wout[:, 1024 * t + 512 * ec:
                                 1024 * t + 512 * ec + 512],
                            start=(t == 0), stop=(t == 1))
                ob = work.tile([128, 1024], F16, tag="ob", bufs=4,
                               name=f"obf{sb}")
                nc.scalar.copy(ob[:, 0:512], psb[:, 0:512])
                nc.vector.tensor_copy(out=ob[:, 512:1024],
                                      in_=psb[:, 512:1024])
                nc.scalar.dma_start(
                    out=out_d[128 * sb:128 * sb + 128, 0:512],
                    in_=ob[:, 0:512])
                nc.sync.dma_start(
                    out=out_d[128 * sb:128 * sb + 128, 512:1024],
                    in_=ob[:, 512:1024])

    nc.compile()
    return nc


_CACHE = {}


def _get_nc():
    if "nc" not in _CACHE:
        _CACHE["nc"] = build_bass()
    return _CACHE["nc"]


def _pack8(a):
    """[1024, X] fp32 -> [128, 8*X] fp16 with E-chunk e at cols X*e."""
    X = a.shape[1]
    return np.ascontiguousarray(
        np.asarray(a, np.float16).reshape(8, 128, X).transpose(1, 0, 2)
        .reshape(128, 8 * X))


def _pack8p(a):
    """[1024, 256] fp32 -> [128, 2048] fp16, p-major: col 1024p + 128e
    holds E-chunk e of pair-half p (cols 128p..128p+128 of the source)."""
    w = np.asarray(a, np.float16).reshape(8, 128, 2, 128)  # e, E, p, col
    return np.ascontiguousarray(
        w.transpose(1, 2, 0, 3).reshape(128, 2048))


def make_core_inputs(x, W_qkv, b_qkv, W_out, b_out):
    """Host-side sharding: returns in_maps for the 8 cores."""
    x = np.asarray(x, np.float32)
    W_qkv = np.asarray(W_qkv, np.float32)
    b_qkv = np.asarray(b_qkv, np.float32)
    W_out = np.asarray(W_out, np.float32)

    mask128 = np.where(
        np.arange(128)[:, None] <= np.arange(128)[None, :],
        np.float32(NEG), np.float32(0)).astype(np.float32)
    onesp = np.ones((128, 64), np.float16)

    xts = [_pack8(x[b].T) for b in range(B)]

    in_maps = []
    for c in range(NCORES):
        b = c // 4
        hg = c % 4
        heads = [4 * hg + l for l in range(4)]
        qcols = np.array([192 * hh + d for hh in heads for d in range(64)])
        cf = np.empty((128, 130), np.float32)
        cf[:, 0:128] = mask128
        cf[:, 128:130] = (b_qkv[qcols] * 0.125).reshape(2, 128).T
        in_maps.append({
            "xt": xts[b],
            "wq": _pack8p(W_qkv[:, qcols] * 0.125),
            "wk": _pack8p(W_qkv[:, qcols + 64]),
            "wv": _pack8(W_qkv[:, qcols + 128]),
            "wout": np.ascontiguousarray(np.asarray(np.hstack([
                W_out[64 * heads[0]:64 * heads[0] + 128],
                W_out[64 * heads[2]:64 * heads[2] + 128]]), np.float16)),
            "cf": cf,
            "onesp": onesp,
        })
    return in_maps


def assemble_output(results, x, W_qkv, b_qkv, W_out, b_out):
    x = np.asarray(x, np.float32)
    W_qkv = np.asarray(W_qkv, np.float32)
    b_qkv = np.asarray(b_qkv, np.float32)
    W_out = np.asarray(W_out, np.float32)
    b_out = np.asarray(b_out, np.float32)

    # v-bias folded into the output bias: attn rows sum to 1, so dropping
    # bv from v shifts vals by exactly bv -> out by bv @ W_out
    bv_vals = np.empty(E, np.float32)
    for hh in range(NH):
        bv_vals[64 * hh:64 * hh + 64] = b_qkv[192 * hh + 128:192 * hh + 192]
    b_out_eff = b_out + bv_vals @ W_out

    out = np.zeros((B, S, E), np.float32)
    for c in range(NCORES):
        out[c // 4] += np.asarray(results[c]["out"], np.float32)
    out += b_out_eff[None, None, :]

    # row S-1: reference's mask makes attention exactly uniform over all keys
    for b in range(B):
        xm = x[b].mean(axis=0)
        vmean = np.empty(E, np.float32)
        for hh in range(NH):
            cols = slice(192 * hh + 128, 192 * hh + 192)
            vmean[64 * hh:64 * hh + 64] = xm @ W_qkv[:, cols] + b_qkv[cols]
        out[b, S - 1] = vmean @ W_out + b_out
    return out


def kernel(x, W_qkv, b_qkv, W_out, b_out, _trace=False):
    nc = _get_nc()
    in_maps = make_core_inputs(x, W_qkv, b_qkv, W_out, b_out)
    res = run_bass_kernel_spmd(nc, in_maps, list(range(NCORES)), trace=_trace)
    out = assemble_output(res.results, x, W_qkv, b_qkv, W_out, b_out)
    if _trace:
        _CACHE["last_results"] = res
    return out



# revision 22
# speedup vs baseline: 1.0389x; 1.0389x over previous
"""Multi-head attention Trainium2 kernel (8 NeuronCores, SPMD).

Problem: nn_MultiHeadAttention (B=2, S=2048, E=1024, H=16, hd=64), fp32 I/O,
reference masks j <= i with -1e9 (attends strictly to the future); row S-1
degenerates to uniform attention and is patched on host.

Sharding: core c handles batch b = c//4 and 4 heads [4*(c%4), 4*(c%4)+4).
Each core computes a partial output [S, E] (its heads' contribution to the
output projection); host sums the 4 partials per batch and adds the bias.

Design (evolved from the 153 us fp16 version):
  - q/k projections run fp8e4 DoubleRow: K=256 per matmul (2 fp8/cell), so
    each 512-col chain is 4 matmuls instead of 8 (measured 234 ns/MM vs
    219 ns for fp16 K=128 -> ~1.9x on the q/k chains). Weights carry exact
    power-of-2 scales (q: x512 on top of the 0.125 softmax scale, k: x256)
    to center W~N(0,0.02) in fp8 range; the eviction multiplies them back
    out ((ps*(1/512))+bias via scalar_tensor_tensor, ps*(1/256) via
    tensor_scalar). x itself is ~N(0,1): quantizes to fp8 directly.
    v / scores / attn@v / out-proj stay fp16 (v and vals feed the output
    directly; fp8 there costs ~3% of scale vs ~0.1% via the softmax path).
  - zero bias matmuls: k-bias dropped (softmax is invariant to per-query
    constants), q-bias fused into the qT eviction, v-bias folded into
    b_out on host (attn rows sum to 1).
  - scores per head-pair: two K=64 matmuls on array row-groups 0/64 (these
    co-issue: measured 111 ns/MM for the pair pattern); attn@v col-packed:
    head X -> PSUM partitions 64X..64X+63, concurrent via col groups.
  - softmax denominators as M=64 ones-matmuls writing 64 identical rows
    per head into dn partitions 0-63/64-127: the partition broadcast IS
    the matmul, so normalize is reciprocal_approx_fast + one multiply.
  - exp on the scalar engine reads score pairs packed [0 : w0+w1] in one
    PSUM tile; diagonal masking is one fused dual-window DVE add using a
    stride-0 repeat of a [128,128] mask constant.
  - ~60 warmup matmuls on a memset tile run during the ~8 us instruction
    static-load + first-DMA window: they lift the PE HAM clock gate to
    8/8 before the first real chain and keep it there.
  - schedule: i-chunks processed 3->0 with jb descending; projection and
    out-proj units drip into the attention stream as fillers so PE work
    covers the scalar engine's exp; PSUM = ring 3x[128,1024] + va + dn.
  - DMA: wq8/xt8-sc3 first (smallest eager set), issue split across the
    Sync and Scalar HW-DGE queues; xt8 seq-chunks land per-E-chunk so DR
    accumulation steps start as chunks arrive; final out-proj interleaves
    per-half evictions and stores across engines and queues.
"""

from collections import deque

import numpy as np

import concourse.bacc as bacc
import concourse.bass as bass
import concourse.mybir as mybir
from concourse.tile import TileContext
from concourse.bass_utils import run_bass_kernel_spmd

S = 2048
E = 1024
HD = 64
NH = 16
B = 2
NCORES = 8
NEG = -1000000000.0

F32 = mybir.dt.float32
F16 = mybir.dt.float16
F8 = mybir.dt.float8e4
DR = mybir.MatmulPerfMode.DoubleRow
MULT = mybir.AluOpType.mult
ADD = mybir.AluOpType.add

EXP = mybir.ActivationFunctionType.Exp

QSC = 512.0  # fp8 range scale folded into wq8 (on top of 0.125)
KSC = 256.0  # fp8 range scale folded into wk8


def build_bass():
    nc = bacc.Bacc()

    xt_d = nc.dram_tensor("xt", [128, 8 * S], F16, kind="ExternalInput")
    xt8_d = nc.dram_tensor("xt8", [128, 8 * S], F8, kind="ExternalInput")
    wq8_d = nc.dram_tensor("wq8", [128, 2048], F8, kind="ExternalInput")
    wk8_d = nc.dram_tensor("wk8", [128, 2048], F8, kind="ExternalInput")
    wv_d = nc.dram_tensor("wv", [128, 2048], F16, kind="ExternalInput")
    wout_d = nc.dram_tensor("wout", [128, 2048], F16, kind="ExternalInput")
    cf_d = nc.dram_tensor("cf", [128, 130], F32, kind="ExternalInput")
    onesp_d = nc.dram_tensor("onesp", [128, 64], F16, kind="ExternalInput")
    out_d = nc.dram_tensor("out", [S, E], F16, kind="ExternalOutput")

    with TileContext(nc) as tc:
        with (
            tc.tile_pool(name="big", bufs=1) as big,
            tc.tile_pool(name="small", bufs=1) as small,
            tc.tile_pool(name="work", bufs=1) as work,
            tc.tile_pool(name="psum", bufs=1, space="PSUM") as pp,
        ):
            # ---- static SBUF tensors ----
            xt = big.tile([128, 8 * S], F16, tag="xt", name="xt")
            xt8 = big.tile([128, 8 * S], F8, tag="xt8", name="xt8")
            wq8 = small.tile([128, 2048], F8, tag="wq8", name="wq8")
            wk8 = small.tile([128, 2048], F8, tag="wk8", name="wk8")
            wv = small.tile([128, 2048], F16, tag="wv", name="wv")
            wout = small.tile([128, 2048], F16, tag="wout", name="wout")
            cf = small.tile([128, 130], F32, tag="cf", name="cf")
            onesp = small.tile([128, 64], F16, tag="onesp", name="onesp")
            # qT/kT tiles [128 = 2 heads x 64 dims, S]; index = 2*pair + qk
            qkT = [big.tile([128, S], F16, tag="qkT", bufs=4, name=f"qkT{i}")
                   for i in range(4)]
            # v natural [128 seq, 256 = 4 heads x 64]
            v16 = [big.tile([128, 256], F16, tag="v16", bufs=16,
                            name=f"v16_{s}") for s in range(16)]
            # normalized valsT per pair [128 dims, S]
            valsT = [big.tile([128, S], F16, tag="valsT", bufs=2,
                              name=f"valsT{t}") for t in range(2)]

            mask = cf[:, 0:128]

            # ---- warmup: PE busy during instruction-load + first DMAs,
            # so HAM reaches K=8/8 before the first projection chain
            warm = work.tile([128, 640], F16, tag="warm", name="warm")
            nc.vector.memset(warm, 0.0)
            warm_ps = pp.tile([128, 1024], F32, tag="sc", bufs=3,
                              name="warm_ps")
            for i in range(22):
                nc.tensor.matmul(warm_ps[:, 0:128], warm[:, 512:640],
                                 warm[:, 0:128], start=True, stop=True)
            for i in range(4):
                nc.tensor.matmul(warm_ps[:, 0:512], warm[:, 512:640],
                                 warm[:, 0:512], start=True, stop=True)

            # ---- input DMAs: eager set (wq8/wk8 + xt8 sc=3) first, split
            # across the Sync and Scalar HW-DGE queues; xt8 per-E-chunk so
            # DR accumulation steps start as chunks land
            def xt8_sc(c, sc, eng):
                src = bass.AP(tensor=xt8_d, offset=4096 * c + 512 * sc,
                              ap=[[8 * S, 128], [2048, 2], [1, 512]])
                dst = bass.AP(tensor=xt8.tensor,
                              offset=xt8.offset + 4096 * c + 512 * sc,
                              ap=[[xt8.ap[0][0], 128], [2048, 2], [1, 512]])
                eng.dma_start(out=dst, in_=src)

            def xt_quarter(quarter, elo, ehi, eng):
                src = bass.AP(tensor=xt_d, offset=S * elo + 512 * quarter,
                              ap=[[8 * S, 128], [S, ehi - elo], [1, 512]])
                dst = bass.AP(tensor=xt.tensor,
                              offset=xt.offset + S * elo + 512 * quarter,
                              ap=[[xt.ap[0][0], 128], [S, ehi - elo],
                                  [1, 512]])
                eng.dma_start(out=dst, in_=src)

            # issue cost is ~0.65us of engine time per dma_start, so the
            # early set is few + large + ordered by first use; wout rides
            # the otherwise-idle Vector queue
            # Scalar carries only 5 early issues (done before exp starts);
            # Sync takes the critical-path set; gpsimd's software DGE
            # streams the bulk late loads (wout, xt quarters 2/1/0)
            nc.scalar.dma_start(out=wq8[:, 0:256], in_=wq8_d[:, 0:256])
            nc.sync.dma_start(out=cf, in_=cf_d[:, :])
            nc.sync.dma_start(out=onesp, in_=onesp_d[:, :])
            nc.sync.dma_start(out=wk8[:, 0:256], in_=wk8_d[:, 0:256])
            xt8_sc(0, 3, nc.scalar)
            xt8_sc(1, 3, nc.sync)
            nc.scalar.dma_start(out=wq8[:, 256:1024], in_=wq8_d[:, 256:1024])
            nc.sync.dma_start(out=wk8[:, 256:1024], in_=wk8_d[:, 256:1024])
            xt8_sc(2, 3, nc.scalar)
            xt8_sc(3, 3, nc.sync)
            # wv + xt quarter 3 feed the eager v chains (sb 15..12)
            nc.sync.dma_start(out=wv[:, 0:1024], in_=wv_d[:, 0:1024])
            nc.scalar.dma_start(out=wv[:, 1024:2048],
                                in_=wv_d[:, 1024:2048])
            xt_quarter(3, 0, 2, nc.sync)
            xt_quarter(3, 2, 4, nc.sync)
            xt_quarter(3, 4, 6, nc.scalar)
            xt_quarter(3, 6, 8, nc.scalar)
            nc.gpsimd.dma_start(out=wout, in_=wout_d[:, :])
            nc.sync.dma_start(out=wq8[:, 1024:2048],
                              in_=wq8_d[:, 1024:2048])
            nc.sync.dma_start(out=wk8[:, 1024:2048],
                              in_=wk8_d[:, 1024:2048])
            for c in range(4):
                xt8_sc(c, 2, nc.sync)
            xt_quarter(2, 0, 4, nc.sync)
            xt_quarter(2, 4, 8, nc.sync)
            for c in range(4):
                xt8_sc(c, 1, nc.sync)
            xt_quarter(1, 0, 8, nc.sync)
            for c in range(4):
                xt8_sc(c, 0, nc.sync)
            xt_quarter(0, 0, 8, nc.sync)

            # ---- projection / out-proj unit emitters (fillers) ----
            def qk_mms(ps, w8, p, sc):
                for c in range(4):
                    lhsT = bass.AP(
                        tensor=w8.tensor,
                        offset=w8.offset + 1024 * p + 256 * c,
                        ap=[[w8.ap[0][0], 128], [128, 2], [1, 128]])
                    rhs = bass.AP(
                        tensor=xt8.tensor,
                        offset=xt8.offset + 4096 * c + 512 * sc,
                        ap=[[xt8.ap[0][0], 128], [2048, 2], [1, 512]])
                    nc.tensor.matmul(ps, lhsT, rhs, start=(c == 0),
                                     stop=(c == 3), perf_mode=DR)

            def q_chain(p, sc):
                def emit():
                    psb = pp.tile([128, 1024], F32, tag="sc", bufs=3,
                                  name=f"psq{p}{sc}")
                    ps = psb[:, 0:512]
                    qk_mms(ps, wq8, p, sc)
                    bcol = cf[:, 128 + p:129 + p]
                    bb = bass.AP(tensor=bcol.tensor, offset=bcol.offset,
                                 ap=[bcol.ap[0], [0, 512]])
                    nc.vector.scalar_tensor_tensor(
                        out=qkT[2 * p][:, 512 * sc:512 * sc + 512],
                        in0=ps, scalar=1.0 / QSC, in1=bb,
                        op0=MULT, op1=ADD)
                return emit

            def k_chain(p, sc):
                def emit():
                    psb = pp.tile([128, 1024], F32, tag="sc", bufs=3,
                                  name=f"psk{p}{sc}")
                    ps = psb[:, 0:512]
                    qk_mms(ps, wk8, p, sc)
                    nc.vector.tensor_scalar_mul(
                        out=qkT[2 * p + 1][:, 512 * sc:512 * sc + 512],
                        in0=ps, scalar1=1.0 / KSC)
                return emit

            def v_chain(sb):
                def emit():
                    psb = pp.tile([128, 1024], F32, tag="sc", bufs=3,
                                  name=f"psv{sb}")
                    ps = psb[:, 0:256]
                    for e in range(8):
                        nc.tensor.matmul(
                            ps, xt[:, 2048 * e + 128 * sb:2048 * e + 128 * sb + 128],
                            wv[:, 256 * e:256 * e + 256],
                            start=(e == 0), stop=(e == 7))
                    nc.vector.tensor_copy(out=v16[sb], in_=ps)
                return emit

            def outproj_unit(sb, use_act=False, dma_eng=None):
                def emit():
                    psb = pp.tile([128, 1024], F32, tag="sc", bufs=3,
                                  name=f"pso{sb}")
                    for ec in range(2):
                        for t in range(2):
                            nc.tensor.matmul(
                                psb[:, 512 * ec:512 * ec + 512],
                                valsT[t][:, 128 * sb:128 * sb + 128],
                                wout[:, 1024 * t + 512 * ec:
                                     1024 * t + 512 * ec + 512],
                                start=(t == 0), stop=(t == 1))
                    ob = work.tile([128, 1024], F16, tag="ob", bufs=4,
                                   name=f"ob{sb}")
                    if use_act:
                        nc.scalar.copy(ob, psb)
                    else:
                        nc.vector.tensor_copy(out=ob, in_=psb)
                    (dma_eng or nc.sync).dma_start(
                        out=out_d[128 * sb:128 * sb + 128, :], in_=ob)
                return emit

            fillers = deque()

            def emit_fillers(n):
                for _ in range(n):
                    if fillers:
                        fillers.popleft()()

            # eager: just enough projection for attention (ic3, p0) pair 0;
            # the rest of ic3's inputs lead the ic3 filler queue. Warm
            # matmuls interleave with the first chains' accumulation steps
            # so the PE stays busy (HAM 8/8) while their chunks dribble in.
            def qk_mms_warm(ps, w8, p, sc):
                for c in range(4):
                    lhsT = bass.AP(
                        tensor=w8.tensor,
                        offset=w8.offset + 1024 * p + 256 * c,
                        ap=[[w8.ap[0][0], 128], [128, 2], [1, 128]])
                    rhs = bass.AP(
                        tensor=xt8.tensor,
                        offset=xt8.offset + 4096 * c + 512 * sc,
                        ap=[[xt8.ap[0][0], 128], [2048, 2], [1, 512]])
                    nc.tensor.matmul(ps, lhsT, rhs, start=(c == 0),
                                     stop=(c == 3), perf_mode=DR)
                    if c < 3:
                        for _ in range(2):
                            nc.tensor.matmul(
                                warm_ps[:, 512:1024], warm[:, 512:640],
                                warm[:, 0:512], start=True, stop=True)

            psb = pp.tile([128, 1024], F32, tag="sc", bufs=3, name="psq03")
            qk_mms_warm(psb[:, 0:512], wq8, 0, 3)
            bcol = cf[:, 128:129]
            bb = bass.AP(tensor=bcol.tensor, offset=bcol.offset,
                         ap=[bcol.ap[0], [0, 512]])
            nc.vector.scalar_tensor_tensor(
                out=qkT[0][:, 1536:2048], in0=psb[:, 0:512],
                scalar=1.0 / QSC, in1=bb, op0=MULT, op1=ADD)
            psb = pp.tile([128, 1024], F32, tag="sc", bufs=3, name="psk03")
            qk_mms_warm(psb[:, 0:512], wk8, 0, 3)
            nc.vector.tensor_scalar_mul(
                out=qkT[1][:, 1536:2048], in0=psb[:, 0:512],
                scalar1=1.0 / KSC)
            v_chain(15)()
            v_chain(14)()

            # ---- attention, ic descending; fillers drip into the stream ----
            for ic in (3, 2, 1, 0):
                if ic == 3:
                    newf = [v_chain(13), v_chain(12),
                            q_chain(1, 3), k_chain(1, 3)]
                    newf += [q_chain(p, 2) for p in range(2)]
                    newf += [k_chain(p, 2) for p in range(2)]
                    newf += [v_chain(sb) for sb in range(8, 12)]
                elif ic == 2:
                    newf = [q_chain(p, 1) for p in range(2)]
                    newf += [k_chain(p, 1) for p in range(2)]
                    newf += [v_chain(sb) for sb in range(4, 8)]
                    newf += [outproj_unit(sb) for sb in range(12, 16)]
                elif ic == 1:
                    newf = [q_chain(p, 0) for p in range(2)]
                    newf += [outproj_unit(sb) for sb in range(8, 12)]
                else:
                    newf = [k_chain(p, 0) for p in range(2)]
                    newf += [v_chain(sb) for sb in range(0, 4)]
                    newf += [outproj_unit(sb) for sb in range(4, 8)]
                fillers.extend(newf)

                i0 = 512 * ic
                for p in range(2):
                    qTp = qkT[2 * p]
                    kTp = qkT[2 * p + 1]
                    jbs = list(range(15, 4 * ic - 1, -1))  # descending
                    pairs = [(jbs[2 * i], jbs[2 * i + 1])
                             for i in range(len(jbs) // 2)]
                    first_jb = jbs[0]
                    last_jb = jbs[-1]
                    va = pp.tile([128, 512], F32, tag="va", bufs=1,
                                 name=f"va{p}{ic}")
                    dn = pp.tile([128, 512], F32, tag="dn", bufs=1,
                                 name=f"dn{p}{ic}")

                    def emit_attnv(jbpair, pts, widths, offs):
                        for half, jb in enumerate(jbpair):
                            w = widths[half]
                            o = offs[half]
                            st = (jb == first_jb)
                            sp = (jb == last_jb)
                            for X in range(2):
                                nc.tensor.matmul(
                                    va[64 * X:64 * X + 64, 0:w],
                                    v16[jb][:, 64 * (2 * p + X):
                                            64 * (2 * p + X) + 64],
                                    pts[X][:, o:o + w],
                                    start=st, stop=sp)
                            # denominators: 64 identical rows per head so
                            # the partition broadcast IS the matmul
                            for X in range(2):
                                nc.tensor.matmul(
                                    dn[64 * X:64 * X + 64, 0:w],
                                    onesp[:, 0:64],
                                    pts[X][:, o:o + w],
                                    start=st, stop=sp)

                    prev = None
                    for jb0, jb1 in pairs:
                        sc_ps = [pp.tile([128, 1024], F32, tag="sc", bufs=3,
                                         name=f"sc{p}{ic}{jb0}{X}")
                                 for X in range(2)]
                        pts = [work.tile([128, 1024], F16, tag="pt", bufs=4,
                                         name=f"pt{p}{ic}{jb0}{X}")
                               for X in range(2)]
                        # half 1 packed right after half 0 (cols w0..w0+w1):
                        # the exp call covers exactly w0+w1 useful columns
                        w0 = min(128 * jb0 - i0 + 128, 512)
                        w1 = min(128 * jb1 - i0 + 128, 512)
                        ws = [w0, w1]
                        offs = [0, w0]
                        for half, jb in enumerate((jb0, jb1)):
                            j0 = 128 * jb
                            for X in range(2):
                                nc.tensor.matmul(
                                    sc_ps[X][:, offs[half]:offs[half] + ws[half]],
                                    kTp[64 * X:64 * X + 64, j0:j0 + 128],
                                    qTp[64 * X:64 * X + 64, i0:i0 + ws[half]],
                                    start=True, stop=True)
                        o0 = 128 * jb0 - i0
                        if o0 <= 384:
                            # both halves diagonal: one fused add over the
                            # two 128-wide windows (w0-128 apart); the mask
                            # operand repeats via a stride-0 free dim
                            for X in range(2):
                                reg = sc_ps[X][:, o0:o0 + 128]
                                dual = bass.AP(
                                    tensor=reg.tensor, offset=reg.offset,
                                    ap=[reg.ap[0], [w0 - 128, 2], [1, 128]])
                                m2 = bass.AP(
                                    tensor=mask.tensor, offset=mask.offset,
                                    ap=[mask.ap[0], [0, 2], [1, 128]])
                                nc.vector.tensor_add(
                                    out=dual, in0=dual, in1=m2)
                        for X in range(2):
                            nc.scalar.activation(
                                pts[X][:, 0:w0 + w1],
                                sc_ps[X][:, 0:w0 + w1], EXP)
                        if prev is not None:
                            emit_attnv(*prev)
                        emit_fillers(2 if ic >= 2 else 1)
                        prev = ((jb0, jb1), pts, ws, offs)
                    # one filler covers the last pair's exp latency (no
                    # further scores follow to keep the PE busy here)
                    emit_fillers(1)
                    if prev is not None:
                        emit_attnv(*prev)

                    # normalize: reciprocal of the (already broadcast)
                    # denominators, then one partition-aligned multiply
                    rcpt = work.tile([128, 512], F32, tag="rcpt", bufs=2,
                                     name=f"rcpt{p}{ic}")
                    nc.vector.reciprocal_approx_fast(out=rcpt, in_=dn)
                    nc.vector.tensor_mul(
                        out=valsT[p][:, i0:i0 + 512], in0=va, in1=rcpt)

            # flush remaining fillers + final out-proj chunk: per-half
            # evictions and stores interleaved with the matmuls, split
            # across Scalar/Vector and both DMA queues to compress the tail
            emit_fillers(len(fillers))
            for sb in range(0, 4):
                psb = pp.tile([128, 1024], F32, tag="sc", bufs=3,
                              name=f"psf{sb}")
                ob = work.tile([128, 1024], F16, tag="ob", bufs=4,
                               name=f"obf{sb}")
                for ec in range(2):
                    for t in range(2):
                        nc.tensor.matmul(
                            psb[:, 512 * ec:512 * ec + 512],
                            valsT[t][:, 128 * sb:128 * sb + 128],
                            wout[:, 1024 * t + 512 * ec:
                                 1024 * t + 512 * ec + 512],
                            start=(t == 0), stop=(t == 1))
                    lo, hi = 512 * ec, 512 * ec + 512
                    if ec == 0:
                        nc.scalar.copy(ob[:, lo:hi], psb[:, lo:hi])
                        nc.scalar.dma_start(
                            out=out_d[128 * sb:128 * sb + 128, lo:hi],
                            in_=ob[:, lo:hi])
                    else:
                        nc.vector.tensor_copy(out=ob[:, lo:hi],
                                              in_=psb[:, lo:hi])
                        nc.sync.dma_start(
                            out=out_d[128 * sb:128 * sb + 128, lo:hi],
                            in_=ob[:, lo:hi])

    nc.compile()
    return nc


_CACHE = {}


def _get_nc():
    if "nc" not in _CACHE:
        _CACHE["nc"] = build_bass()
    return _CACHE["nc"]


def _pack8(a):
    """[1024, X] fp32 -> [128, 8*X] fp16 with E-chunk e at cols X*e."""
    X = a.shape[1]
    return np.ascontiguousarray(
        np.asarray(a, np.float16).reshape(8, 128, X).transpose(1, 0, 2)
        .reshape(128, 8 * X))


F8NP = mybir.dt.np(F8)


def _pack_xt8(x):
    """[S, E] fp32 -> [128, 8S] fp8 DR layout: partition ki, col
    4096c + 2048ko + s holds x[s, 256c + 128ko + ki]."""
    y = np.clip(x.T, -240, 240).reshape(4, 2, 128, S)  # [c, ko, ki, s]
    return np.ascontiguousarray(
        y.transpose(2, 0, 1, 3).reshape(128, 8 * S).astype(F8NP))


def _pack_w8(w):
    """[1024, 256] fp32 (scaled) -> [128, 2048] fp8 DR layout: partition
    ki, col 1024p + 256c + 128ko + m holds w[256c + 128ko + ki, 128p + m]."""
    z = np.clip(w, -240, 240).reshape(4, 2, 128, 2, 128)  # [c,ko,ki,p,m]
    return np.ascontiguousarray(
        z.transpose(2, 3, 0, 1, 4).reshape(128, 2048).astype(F8NP))


def make_core_inputs(x, W_qkv, b_qkv, W_out, b_out):
    """Host-side sharding: returns in_maps for the 8 cores."""
    x = np.asarray(x, np.float32)
    W_qkv = np.asarray(W_qkv, np.float32)
    b_qkv = np.asarray(b_qkv, np.float32)
    W_out = np.asarray(W_out, np.float32)

    mask128 = np.where(
        np.arange(128)[:, None] <= np.arange(128)[None, :],
        np.float32(NEG), np.float32(0)).astype(np.float32)
    onesp = np.ones((128, 64), np.float16)

    xts = [_pack8(x[b].T) for b in range(B)]
    xt8s = [_pack_xt8(x[b]) for b in range(B)]

    in_maps = []
    for c in range(NCORES):
        b = c // 4
        hg = c % 4
        heads = [4 * hg + l for l in range(4)]
        qcols = np.array([192 * hh + d for hh in heads for d in range(64)])
        cf = np.empty((128, 130), np.float32)
        cf[:, 0:128] = mask128
        cf[:, 128:130] = (b_qkv[qcols] * 0.125).reshape(2, 128).T
        in_maps.append({
            "xt": xts[b],
            "xt8": xt8s[b],
            "wq8": _pack_w8(W_qkv[:, qcols] * (0.125 * QSC)),
            "wk8": _pack_w8(W_qkv[:, qcols + 64] * KSC),
            "wv": _pack8(W_qkv[:, qcols + 128]),
            "wout": np.ascontiguousarray(np.asarray(np.hstack([
                W_out[64 * heads[0]:64 * heads[0] + 128],
                W_out[64 * heads[2]:64 * heads[2] + 128]]), np.float16)),
            "cf": cf,
            "onesp": onesp,
        })
    return in_maps


def assemble_output(results, x, W_qkv, b_qkv, W_out, b_out):
    x = np.asarray(x, np.float32)
    W_qkv = np.asarray(W_qkv, np.float32)
    b_qkv = np.asarray(b_qkv, np.float32)
    W_out = np.asarray(W_out, np.float32)
    b_out = np.asarray(b_out, np.float32)

    # v-bias folded into the output bias: attn rows sum to 1, so dropping
    # bv from v shifts vals by exactly bv -> out by bv @ W_out
    bv_vals = np.empty(E, np.float32)
    for hh in range(NH):
        bv_vals[64 * hh:64 * hh + 64] = b_qkv[192 * hh + 128:192 * hh + 192]
    b_out_eff = b_out + bv_vals @ W_out

    out = np.zeros((B, S, E), np.float32)
    for c in range(NCORES):
        out[c // 4] += np.asarray(results[c]["out"], np.float32)
    out += b_out_eff[None, None, :]

    # row S-1: reference's mask makes attention exactly uniform over all keys
    for b in range(B):
        xm = x[b].mean(axis=0)
        vmean = np.empty(E, np.float32)
        for hh in range(NH):
            cols = slice(192 * hh + 128, 192 * hh + 192)
            vmean[64 * hh:64 * hh + 64] = xm @ W_qkv[:, cols] + b_qkv[cols]
        out[b, S - 1] = vmean @ W_out + b_out
    return out


def kernel(x, W_qkv, b_qkv, W_out, b_out, _trace=False):
    nc = _get_nc()
    in_maps = make_core_inputs(x, W_qkv, b_qkv, W_out, b_out)
    res = run_bass_kernel_spmd(nc, in_maps, list(range(NCORES)), trace=_trace)
    out = assemble_output(res.results, x, W_qkv, b_qkv, W_out, b_out)
    if _trace:
        _CACHE["last_results"] = res
    return out
